# revision 24
# baseline (speedup 1.0000x reference)
"""AttentivePooling Trainium2 kernel.

Computes, per example b:
    h      = tanh(x[b] @ W1 + b1)          # (S, MID)
    scores = h @ w2 (+ b2, dropped: softmax-invariant)
    attn   = softmax(scores)               # over S
    out[b] = attn @ x[b]                   # (C,)

Sharding: batch (32) split across 8 NeuronCores -> 4 examples/core.
Weights replicated. No cross-core communication.

Per-core dataflow (on-chip tensors bf16, accumulation fp32):
  - x loaded HBM->SBUF once, fp32->bf16 cast during DMA (SWDGE),
    natural layout [s=128 partitions, (tile, c) free].
  - xT via TensorE transpose-mode matmuls ([128,128] blocks, identity
    stationary) into bf16 PSUM tiles, drained to SBUF by VectorE copies.
    (The DMA xbar transpose path is unusable here: Tile globally
    serializes xbar transposes against normal DMAs, and the XPOSE ISA
    slot only carries one semaphore wait -- walrus rejects the 2-3 waits
    a pipelined kernel needs.)
  - mm1: hT[m, s] = W1k0.T @ xT[...,half0] + W1k1.T @ xT[...,half1] (PE)
  - tanh(. + b1) via ScalarE per-partition bias, bf16 out.
  - mm2: scoresT[:, jj] = h_chunk.T @ w2 -> scores in [128, 64] layout
    (seq position on partitions, seq tile as column) feeding both the
    softmax and mm4's stationary operand with no reshape.
  - softmax WITHOUT max subtraction: |scores| <= ||w2||_1 + |b2| ~ 5.7,
    exp is safe in fp32. exp via ScalarE with fused row-sum accum_out;
    cross-partition total via a ones-vector matmul; normalization
    deferred to the end (scale by 1/sum).
  - mm4: out[1, C] += p[:, t].T @ x_tile over 64 seq tiles, interleaved
    into the NEXT example's score phase so the in-order PE never stalls
    at example boundaries.
"""

from contextlib import ExitStack

import numpy as np

import concourse.bass as bass
import concourse.tile as tile
from concourse import mybir
from concourse.bass_utils import run_bass_kernel_spmd

B, S, C, MID = 32, 8192, 256, 128
N_CORES = 8
B_LOC = B // N_CORES  # 4 examples per core

F32 = mybir.dt.float32
BF16 = mybir.dt.bfloat16
F8 = mybir.dt.float8e4
AF = mybir.ActivationFunctionType
DR = mybir.MatmulPerfMode.DoubleRow

S_TILES = S // 128  # 64 seq tiles of 128 per example
CH = 8  # chunks per example (1024 seq each)
T_PER_CH = S_TILES // CH  # 8 seq tiles per chunk


def build_nc(reps: int = 1, strip_waits: bool = True, mode: str = "full") -> bass.Bass:
    """Build the per-core program. reps>1 repeats the whole computation
    back-to-back inside one NEFF (used only for benchmarking: the wall-time
    difference between reps=R and reps=1 isolates kernel time from dispatch
    overhead). strip_waits=False keeps Tile's full (redundant) semaphore
    waits -- required for CoreSim, whose race detector doesn't model
    engine-FIFO-implied ordering; hardware builds need the strip because
    walrus allows only one sync wait per engine ISA instruction."""
    nc = bass.Bass("TRN2", target_bir_lowering=False, debug=False)

    x_ext = nc.declare_dram_parameter("x", [B_LOC, S, C], F32, isOutput=False)
    w1_ext = nc.declare_dram_parameter("W1", [C, MID], F32, isOutput=False)
    b1_ext = nc.declare_dram_parameter("b1", [MID], F32, isOutput=False)
    w2_ext = nc.declare_dram_parameter("w2", [MID], F32, isOutput=False)
    id_ext = nc.declare_dram_parameter("ident", [128, 128], BF16, isOutput=False)
    out_ext = nc.declare_dram_parameter("out", [B_LOC, C], F32, isOutput=True)

    with tile.TileContext(nc) as tc, ExitStack() as ctx:
        const_pool = ctx.enter_context(tc.tile_pool(name="const", bufs=1))
        xnat_pool = ctx.enter_context(tc.tile_pool(name="xnat", bufs=3))
        xt_pool = ctx.enter_context(tc.tile_pool(name="xt", bufs=3))
        ht_pool = ctx.enter_context(tc.tile_pool(name="ht", bufs=4))
        small_pool = ctx.enter_context(tc.tile_pool(name="small", bufs=4))
        psum_xt = ctx.enter_context(tc.tile_pool(name="psum_xt", bufs=2, space="PSUM"))
        psum_ht = ctx.enter_context(tc.tile_pool(name="psum_ht", bufs=2, space="PSUM"))
        psum_sc = ctx.enter_context(tc.tile_pool(name="psum_sc", bufs=2, space="PSUM"))

        # ---- constants (one-time, tiny) ----
        # W1 stays bf16: its quantization error is systematic across all seq
        # positions (same perturbed weights for every score), so unlike
        # per-position x noise it does NOT average out in the softmax-weighted
        # sum -- fp8 W1 measured 2.2e-2 rel err vs 4.5e-3 for fp8 x.
        w1_sb = const_pool.tile([128, 2, MID], BF16, tag="w1")
        nc.gpsimd.dma_start(
            out=w1_sb[:], in_=w1_ext[:].rearrange("(k p) m -> p k m", p=128)
        )
        w2_sb = const_pool.tile([128, 1], BF16, tag="w2")
        nc.gpsimd.dma_start(out=w2_sb[:], in_=w2_ext[:].rearrange("(p o) -> p o", o=1))
        b1_sb = const_pool.tile([128, 1], F32, tag="b1")
        nc.gpsimd.dma_start(out=b1_sb[:], in_=b1_ext[:].rearrange("(p o) -> p o", o=1))
        ones_bf = const_pool.tile([128, 1], BF16, tag="ones_bf")
        nc.vector.memset(ones_bf[:], 1.0)
        ones_sb = const_pool.tile([128, 1], F32, tag="ones")
        nc.vector.memset(ones_sb[:], 1.0)
        ident_sb = const_pool.tile([128, 128], BF16, tag="ident")
        nc.gpsimd.dma_start(out=ident_sb[:], in_=id_ext[:])
        # [128, 64] tile holding I64 in BOTH partition halves: the rhs of a
        # 64x64 quadrant transpose must be I64 at the quadrant's base
        # partition (walrus: lhsT.base_partition == rhs.base_partition).
        ident2 = const_pool.tile([128, 64], BF16, tag="ident2")
        nc.gpsimd.dma_start(out=ident2[0:64, :], in_=id_ext[0:64, 0:64])
        nc.gpsimd.dma_start(out=ident2[64:128, :], in_=id_ext[0:64, 0:64])
        # Warm up the ScalarE activation table (exp_and_others: tanh+exp) on a
        # tiny input. Walrus attaches the table-load sync to the first
        # ACTIVATE; without this, that instruction exceeds the ISA's
        # 2-sync-wait budget once Tile's own deps are added.
        warm_sb = const_pool.tile([128, 1], F32, tag="warm")
        nc.scalar.activation(warm_sb[:], b1_sb[:], AF.Tanh, bias=b1_sb[:])
        # Warm the other engines too, and -- crucially -- make the PE observe
        # every constant's DMA-lane semaphore via one-wait warmup matmuls, so
        # no steady-state matmul ever needs a second (constant-load) wait.
        # Walrus enforces at most ONE sync wait per engine ISA instruction.
        warm2_sb = const_pool.tile([128, 1], F32, tag="warm2")
        nc.vector.tensor_copy(warm2_sb[:], ones_sb[:])
        warm_ps = psum_ht.tile([128, 512], F32, tag="hT")
        nc.tensor.matmul(
            warm_ps[0:1, 0:1], ones_sb[:], ones_sb[:], start=True, stop=True
        )
        nc.tensor.matmul(
            warm_ps[0:1, 0:1], ident_sb[:, 0:1], ones_bf[:], start=True, stop=True
        )
        nc.tensor.matmul(
            warm_ps[0:1, 0:1], w1_sb[:, 0, 0:1], ones_bf[:], start=True, stop=True
        )
        nc.tensor.matmul(
            warm_ps[0:1, 0:1], ones_bf[:], w2_sb[:], start=True, stop=True
        )
        # warm the quadrant-transpose path so steady-state transposes carry
        # no ident2-DMA wait (one sync wait per ISA instruction).
        warm_xt = psum_xt.tile([128, T_PER_CH * 128], BF16, tag="ps_xt0")
        nc.tensor.transpose(warm_xt[0:64, 0:64], ident_sb[0:64, 0:64], ident2[0:64, :])
        nc.tensor.transpose(
            warm_xt[0:64, 64:128], ident_sb[64:128, 0:64], ident2[64:128, :]
        )

        def emit_mm4_slice(st, lo, hi):
            """Accumulate seq tiles [lo, hi) of a pending example into outacc."""
            for t in range(lo, hi):
                nc.tensor.matmul(
                    st["outacc"][:],
                    st["p_bf"][:, t : t + 1],
                    st["x_ch"][:, t, :],
                    start=(t == 0),
                    stop=(t == S_TILES - 1),
                )

        def finish_phase_b(st):
            out_sb = small_pool.tile([1, C], F32, tag="out_sb")
            nc.vector.tensor_scalar_mul(out_sb[:], st["outacc"][:], st["recip"][:])
            nc.scalar.dma_start(out=out_ext[st["b"] : st["b"] + 1, :], in_=out_sb[:])

        def emit_mm1_tanh(st2, ch):
            """mm1 + tanh for chunk ch (emitted one chunk late: the xt drains
            and the WAR-cleared hT PSUM slots are then a full chunk old, so
            the in-order PE queue never stalls on DVE/ScalarE latency)."""
            xt0, xt1 = st2["xts"][ch]
            hts = []
            for sub in range(T_PER_CH * 128 // 512):
                s0 = sub * 512
                hT_ps = psum_ht.tile([128, 512], F32, tag="hT")
                nc.tensor.matmul(
                    hT_ps[:], w1_sb[:, 0, :], xt0[:, s0 : s0 + 512],
                    start=True, stop=False,
                )
                nc.tensor.matmul(
                    hT_ps[:], w1_sb[:, 1, :], xt1[:, s0 : s0 + 512],
                    start=False, stop=True,
                )
                h_t = ht_pool.tile([128, 512], BF16, tag="ht")
                nc.scalar.activation(h_t[:], hT_ps[:], AF.Tanh, bias=b1_sb[:])
                hts.append(h_t)
            st2["hts"][ch] = hts

        def emit_mm2(st2, ch):
            """scores for chunk ch (a further half-chunk later, so each tanh
            had a whole chunk of ScalarE slack before its mm2 reads it)."""
            for sub in range(T_PER_CH * 128 // 512):
                h_t = st2["hts"][ch][sub]
                for j in range(4):
                    jj = ch * T_PER_CH + sub * 4 + j
                    nc.tensor.matmul(
                        st2["scoresT_ps"][:, jj : jj + 1],
                        h_t[:, j * 128 : (j + 1) * 128],
                        w2_sb[:],
                        start=True,
                        stop=True,
                    )

        pending = None
        for b in [bb for _ in range(reps) for bb in range(B_LOC)]:
            # One PSUM bank per example holds everything softmax-related:
            # cols [0:64] scoresT, col [64] the exp-sum, cols [65:321] the
            # output accumulator (partition 0 only for the latter two). This
            # keeps every consumer's dependencies on a single semaphore.
            scoresT_ps = psum_sc.tile([128, S_TILES + 1 + C], F32, tag="scoresT")
            # whole example's x in one slot; loaded by CH//2 batched DMAs
            # (2MB each) to amortize descriptor-gen + completion fixed costs.
            x_ch = xnat_pool.tile([128, S_TILES, C], BF16, tag="xnat")
            st2 = {"scoresT_ps": scoresT_ps, "xts": {}, "hts": {}}
            for ch in range(CH):
                # keep the PE busy across example boundaries: weave the
                # previous example's weighted-sum matmuls between chunks.
                # Emitted at chunk TOP so later same-engine waits imply them.
                if mode != "dma":
                    if pending is not None:
                        emit_mm4_slice(pending, ch * T_PER_CH, (ch + 1) * T_PER_CH)
                    if ch > 1:
                        emit_mm2(st2, ch - 2)
                if ch % 2 == 0 and mode != "pe":
                    # load 2048 seq rows, cast fp32->bf16 during DMA
                    t0g = ch * T_PER_CH
                    src = x_ext[b, t0g * 128 : (t0g + 2 * T_PER_CH) * 128, :]
                    nc.gpsimd.dma_start(
                        out=x_ch[:, t0g : t0g + 2 * T_PER_CH, :],
                        in_=src.rearrange("(t p) c -> p t c", p=128),
                    )
                if mode == "dma":
                    continue

                # TensorE transpose of each [128,128] block into bf16 PSUM,
                # per c-half; VectorE drains PSUM -> SBUF (cheap: bf16 2x mode)
                ps_xt0 = psum_xt.tile([128, T_PER_CH * 128], BF16, tag="ps_xt0")
                ps_xt1 = psum_xt.tile([128, T_PER_CH * 128], BF16, tag="ps_xt1")
                for t in range(T_PER_CH):
                    # 4x 64x64 quadrant transposes per [128,128] block: the
                    # quadrants land on distinct (row_grp, col_grp) 64-strips
                    # of the PE array and run concurrently (tile_position is
                    # auto-derived from the operand base partitions).
                    for ps, half in ((ps_xt0, 0), (ps_xt1, 1)):
                        for si in range(2):
                            for cj in range(2):
                                nc.tensor.transpose(
                                    ps[
                                        64 * cj : 64 * cj + 64,
                                        t * 128 + 64 * si : t * 128 + 64 * si + 64,
                                    ],
                                    x_ch[
                                        64 * si : 64 * si + 64,
                                        ch * T_PER_CH + t,
                                        half * 128 + 64 * cj : half * 128 + 64 * cj + 64,
                                    ],
                                    ident2[64 * si : 64 * si + 64, :],
                                )
                xt0 = xt_pool.tile([128, T_PER_CH * 128], BF16, tag="xt0")
                xt1 = xt_pool.tile([128, T_PER_CH * 128], BF16, tag="xt1")
                nc.vector.tensor_copy(xt0[:], ps_xt0[:])
                nc.vector.tensor_copy(xt1[:], ps_xt1[:])
                st2["xts"][ch] = (xt0, xt1)

                if ch > 0:
                    emit_mm1_tanh(st2, ch - 1)

                if pending is not None and ch == CH - 1:
                    finish_phase_b(pending)
                    pending = None

            if mode == "dma":
                continue
            # ---- tail: last chunk's scores, then softmax reductions ----
            emit_mm1_tanh(st2, CH - 1)
            emit_mm2(st2, CH - 2)
            emit_mm2(st2, CH - 1)
            p_bf = small_pool.tile([128, S_TILES], BF16, tag="p")
            sumrow = small_pool.tile([128, 1], F32, tag="sumrow")
            nc.scalar.activation(
                p_bf[:], scoresT_ps[:, 0:S_TILES], AF.Exp, accum_out=sumrow[:]
            )
            # cross-partition exp-sum lands in the scores tile's spare column
            # (same PSUM bank -> no extra slot, and the matmul's only wait is
            # the ScalarE accum above)
            nc.tensor.matmul(
                scoresT_ps[0:1, S_TILES : S_TILES + 1],
                ones_sb[:],
                sumrow[:],
                start=True,
                stop=True,
            )
            # give the DVE an up-to-date ScalarE observation so the
            # reciprocal's only explicit wait is the PE (sum matmul)
            dve_obs = small_pool.tile([1, 1], BF16, tag="dve_obs")
            nc.vector.tensor_copy(dve_obs[:], p_bf[0:1, 0:1])
            recip = small_pool.tile([1, 1], F32, tag="recip")
            nc.vector.reciprocal(recip[:], scoresT_ps[0:1, S_TILES : S_TILES + 1])
            pending = {
                "b": b,
                "p_bf": p_bf,
                "x_ch": x_ch,
                "recip": recip,
                "outacc": scoresT_ps[0:1, S_TILES + 1 : S_TILES + 1 + C],
            }

        # tail: last example's weighted sum
        if pending is not None:
            emit_mm4_slice(pending, 0, S_TILES)
            finish_phase_b(pending)

    if strip_waits:
        _strip_implied_self_waits(nc)
    return nc


def _strip_implied_self_waits(nc: bass.Bass) -> None:
    """Reduce per-instruction sync waits to what the hardware needs.

    Walrus accepts at most ONE sync wait per engine ISA instruction, but
    Tile emits waits per logical dependency. Two sound reductions:

    1. Engine-clock elision. Each engine's sequencer evaluates waits in
       program order and engines retire in order, so if an earlier
       instruction on the SAME engine already waited for sem >= v' (v'>=v),
       a later instruction's wait for sem >= v is vacuous: the semaphore
       condition held before the predecessor issued. (Tile deliberately
       doesn't do this transitive per-proc minimization.) Also covers waits
       on the engine's own completion semaphore.

    2. x-load WAW elision. Each x-chunk load carries {PE >= k (WAR: all
       readers of the slot's old contents are done), DMASW >= v (WAW vs the
       old writer)}. The readers read-after-wrote the old data, so the WAR
       wait transitively dominates the WAW wait; drop the DMASW wait.
    """
    eng_prefix = {
        mybir.EngineType.PE: "PE_",
        mybir.EngineType.DVE: "DVE_",
        mybir.EngineType.Activation: "Activation_",
        mybir.EngineType.Pool: "Pool_",
        mybir.EngineType.SP: "SP_",
    }
    # Sems that are ever non-monotonically updated (barrier gather/release
    # use sem-sub) are excluded from all reasoning: their values regress.
    nonmono: set[str] = set()
    for f in nc.m.functions:
        for blk in f.blocks:
            for inst in blk.instructions:
                si = inst.sync_info
                if si is None:
                    continue
                for u in si.on_update:
                    if u.sync_type == "semaphore" and u.update_mode not in (
                        "sem-inc",
                        "sem-add-imm",
                    ):
                        nonmono.add(u.ant_name)

    observed: dict[mybir.EngineType, dict[str, int]] = {}
    for f in nc.m.functions:
        for blk in f.blocks:
            splits: list[tuple[int, list]] = []
            for idx, inst in enumerate(blk.instructions):
                si = inst.sync_info
                if si is None:
                    continue
                tn = type(inst).__name__
                if tn == "InstEventSemaphore":
                    continue  # barrier machinery: leave untouched
                eng = inst.engine
                obs = observed.setdefault(eng, {})
                pref = eng_prefix.get(eng)
                is_x_load = False
                if tn == "InstDMACopy" and eng == mybir.EngineType.Pool:
                    try:
                        is_x_load = "x_ch" in str(inst.outs[0])
                    except Exception:
                        is_x_load = False
                has_pe_wait = any(
                    w.sync_type == "semaphore" and w.ant_name.startswith("PE_")
                    for w in si.on_wait
                )
                kept = []
                for w in si.on_wait:
                    if (
                        w.sync_type != "semaphore"
                        or w.wait_mode != "sem-ge-imm"
                        or w.ant_name in nonmono
                        or tn == "InstDrain"
                    ):
                        kept.append(w)
                        continue
                    # (1) engine-clock / self-wait elision
                    if obs.get(w.ant_name, 0) >= w.wait_value:
                        continue
                    # (2) x-load WAW-vs-old-writer elision
                    if (
                        is_x_load
                        and has_pe_wait
                        and w.ant_name.startswith("DMASW")
                    ):
                        continue
                    kept.append(w)
                # record knowledge from ALL original waits (sound even for
                # stripped ones: the condition held at this program point)
                for w in si.on_wait:
                    if (
                        w.sync_type == "semaphore"
                        and w.wait_mode == "sem-ge-imm"
                        and w.ant_name not in nonmono
                    ):
                        if obs.get(w.ant_name, 0) < w.wait_value:
                            obs[w.ant_name] = w.wait_value
                if len(kept) != len(si.on_wait):
                    si.on_wait = kept
                    kept = si.on_wait  # re-read normalized
                if len(kept) > 1:
                    # Hardware takes one sync wait per instruction: carry the
                    # surplus on single-wait Drain instructions inserted just
                    # before (same engine => sequencer evaluates them first).
                    extras = []
                    for i, w in enumerate(kept[:-1]):
                        d = mybir.InstDrain(
                            name=f"{inst.name}-w{i}", ins=[], outs=[]
                        )
                        d.engine = inst.engine
                        d.sync_info = mybir.SyncInfo(on_wait=[w], on_update=[])
                        extras.append(d)
                    si.on_wait = [kept[-1]]
                    splits.append((idx, extras))
                # engine-own completion increments advance the engine clock.
                # Pool excluded: its 8 Q7 cores may retire out of order, so
                # completion-count knowledge is only valid for strict-FIFO
                # engines (wait-observation inheritance above is still valid
                # for Pool -- the NX sequencer evaluates waits in order).
                if pref is not None and eng != mybir.EngineType.Pool:
                    for u in si.on_update:
                        if (
                            u.sync_type == "semaphore"
                            and u.update_mode in ("sem-inc", "sem-add-imm")
                            and u.ant_name.startswith(pref)
                        ):
                            obs[u.ant_name] = obs.get(u.ant_name, 0) + (
                                u.update_value or 1
                            )
            if splits:
                il = blk.instructions
                for idx, extras in reversed(splits):
                    for d in reversed(extras):
                        il.insert(idx, d)


_NC_CACHE = None


def _get_nc() -> bass.Bass:
    global _NC_CACHE
    if _NC_CACHE is None:
        _NC_CACHE = build_nc()
    return _NC_CACHE


def make_in_maps(x, W1, b1, w2):
    import ml_dtypes

    ident = np.eye(128, dtype=ml_dtypes.bfloat16)
    return [
        {
            "x": x[i * B_LOC : (i + 1) * B_LOC],
            "W1": W1,
            "b1": b1,
            "w2": w2,
            "ident": ident,
        }
        for i in range(N_CORES)
    ]


def kernel(x, W1, b1, w2, b2=None, **_unused) -> np.ndarray:
    """Full-input entry point: shard batch across 8 cores, run, gather.

    b2 is mathematically irrelevant (softmax shift invariance) and ignored.
    """
    x = np.ascontiguousarray(np.asarray(x, dtype=np.float32))
    W1 = np.ascontiguousarray(np.asarray(W1, dtype=np.float32))
    b1 = np.ascontiguousarray(np.asarray(b1, dtype=np.float32))
    w2 = np.ascontiguousarray(np.asarray(w2, dtype=np.float32))
    assert x.shape == (B, S, C), x.shape

    nc = _get_nc()
    in_maps = make_in_maps(x, W1, b1, w2)
    res = run_bass_kernel_spmd(nc, in_maps, list(range(N_CORES))).results
    out = np.concatenate([res[i]["out"] for i in range(N_CORES)], axis=0)
    return out.astype(np.float32)



# revision 29
# speedup vs baseline: 6.1638x; 6.1638x over previous
"""AttentivePooling Trainium2 kernel.

Computes, per example b:
    h      = tanh(x[b] @ W1 + b1)          # (S, MID)
    scores = h @ w2 (+ b2, dropped: softmax-invariant)
    attn   = softmax(scores)               # over S
    out[b] = attn @ x[b]                   # (C,)

Sharding: batch (32) split across 8 NeuronCores -> 4 examples/core.
Weights replicated. No cross-core communication.

Per-core dataflow (on-chip tensors bf16, accumulation fp32):
  - x loaded HBM->SBUF once, fp32->bf16 cast during DMA (SWDGE),
    natural layout [s=128 partitions, (tile, c) free].
  - xT via TensorE transpose-mode matmuls ([128,128] blocks, identity
    stationary) into bf16 PSUM tiles, drained to SBUF by VectorE copies.
    (The DMA xbar transpose path is unusable here: Tile globally
    serializes xbar transposes against normal DMAs, and the XPOSE ISA
    slot only carries one semaphore wait -- walrus rejects the 2-3 waits
    a pipelined kernel needs.)
  - mm1: hT[m, s] = W1k0.T @ xT[...,half0] + W1k1.T @ xT[...,half1] (PE)
  - tanh(. + b1) via ScalarE per-partition bias, bf16 out.
  - mm2: scoresT[:, jj] = h_chunk.T @ w2 -> scores in [128, 64] layout
    (seq position on partitions, seq tile as column) feeding both the
    softmax and mm4's stationary operand with no reshape.
  - softmax WITHOUT max subtraction: |scores| <= ||w2||_1 + |b2| ~ 5.7,
    exp is safe in fp32. exp via ScalarE with fused row-sum accum_out;
    cross-partition total via a ones-vector matmul; normalization
    deferred to the end (scale by 1/sum).
  - mm4: out[1, C] += p[:, t].T @ x_tile over 64 seq tiles, interleaved
    into the NEXT example's score phase so the in-order PE never stalls
    at example boundaries.
"""

from contextlib import ExitStack

import numpy as np

import concourse.bass as bass
import concourse.tile as tile
from concourse import mybir
from concourse.bass_utils import run_bass_kernel_spmd

B, S, C, MID = 32, 8192, 256, 128
N_CORES = 8
B_LOC = B // N_CORES  # 4 examples per core

F32 = mybir.dt.float32
BF16 = mybir.dt.bfloat16
F8 = mybir.dt.float8e4
AF = mybir.ActivationFunctionType
DR = mybir.MatmulPerfMode.DoubleRow

S_TILES = S // 128  # 64 seq tiles of 128 per example
CH = 8  # chunks per example (1024 seq each)
T_PER_CH = S_TILES // CH  # 8 seq tiles per chunk


def build_nc(reps: int = 1, strip_waits: bool = True, mode: str = "full") -> bass.Bass:
    """Build the per-core program. reps>1 repeats the whole computation
    back-to-back inside one NEFF (used only for benchmarking: the wall-time
    difference between reps=R and reps=1 isolates kernel time from dispatch
    overhead). strip_waits=False keeps Tile's full (redundant) semaphore
    waits -- required for CoreSim, whose race detector doesn't model
    engine-FIFO-implied ordering; hardware builds need the strip because
    walrus allows only one sync wait per engine ISA instruction."""
    nc = bass.Bass("TRN2", target_bir_lowering=False, debug=False)

    x_ext = nc.declare_dram_parameter("x", [B_LOC, S, C], F32, isOutput=False)
    w1_ext = nc.declare_dram_parameter("W1", [C, MID], F32, isOutput=False)
    b1_ext = nc.declare_dram_parameter("b1", [MID], F32, isOutput=False)
    w2_ext = nc.declare_dram_parameter("w2", [MID], F32, isOutput=False)
    id_ext = nc.declare_dram_parameter("ident", [128, 128], BF16, isOutput=False)
    out_ext = nc.declare_dram_parameter("out", [B_LOC, C], F32, isOutput=True)

    with tile.TileContext(nc) as tc, ExitStack() as ctx:
        const_pool = ctx.enter_context(tc.tile_pool(name="const", bufs=1))
        xnat_pool = ctx.enter_context(tc.tile_pool(name="xnat", bufs=3))
        xt_pool = ctx.enter_context(tc.tile_pool(name="xt", bufs=3))
        ht_pool = ctx.enter_context(tc.tile_pool(name="ht", bufs=4))
        small_pool = ctx.enter_context(tc.tile_pool(name="small", bufs=4))
        psum_xt = ctx.enter_context(tc.tile_pool(name="psum_xt", bufs=2, space="PSUM"))
        psum_ht = ctx.enter_context(tc.tile_pool(name="psum_ht", bufs=2, space="PSUM"))
        psum_sc = ctx.enter_context(tc.tile_pool(name="psum_sc", bufs=2, space="PSUM"))

        # ---- constants (one-time, tiny) ----
        # W1 stays bf16: its quantization error is systematic across all seq
        # positions (same perturbed weights for every score), so unlike
        # per-position x noise it does NOT average out in the softmax-weighted
        # sum -- fp8 W1 measured 2.2e-2 rel err vs 4.5e-3 for fp8 x.
        w1_sb = const_pool.tile([128, 2, MID], BF16, tag="w1")
        nc.gpsimd.dma_start(
            out=w1_sb[:], in_=w1_ext[:].rearrange("(k p) m -> p k m", p=128)
        )
        w2_sb = const_pool.tile([128, 1], BF16, tag="w2")
        nc.gpsimd.dma_start(out=w2_sb[:], in_=w2_ext[:].rearrange("(p o) -> p o", o=1))
        b1_sb = const_pool.tile([128, 1], F32, tag="b1")
        nc.gpsimd.dma_start(out=b1_sb[:], in_=b1_ext[:].rearrange("(p o) -> p o", o=1))
        ones_bf = const_pool.tile([128, 1], BF16, tag="ones_bf")
        nc.vector.memset(ones_bf[:], 1.0)
        ones_sb = const_pool.tile([128, 1], F32, tag="ones")
        nc.vector.memset(ones_sb[:], 1.0)
        ident_sb = const_pool.tile([128, 128], BF16, tag="ident")
        nc.gpsimd.dma_start(out=ident_sb[:], in_=id_ext[:])
        # Warm up the ScalarE activation table (exp_and_others: tanh+exp) on a
        # tiny input. Walrus attaches the table-load sync to the first
        # ACTIVATE; without this, that instruction exceeds the ISA's
        # 2-sync-wait budget once Tile's own deps are added.
        warm_sb = const_pool.tile([128, 1], F32, tag="warm")
        nc.scalar.activation(warm_sb[:], b1_sb[:], AF.Tanh, bias=b1_sb[:])
        # Warm the other engines too, and -- crucially -- make the PE observe
        # every constant's DMA-lane semaphore via one-wait warmup matmuls, so
        # no steady-state matmul ever needs a second (constant-load) wait.
        # Walrus enforces at most ONE sync wait per engine ISA instruction.
        warm2_sb = const_pool.tile([128, 1], F32, tag="warm2")
        nc.vector.tensor_copy(warm2_sb[:], ones_sb[:])
        warm_ps = psum_ht.tile([128, 512], F32, tag="hT")
        nc.tensor.matmul(
            warm_ps[0:1, 0:1], ones_sb[:], ones_sb[:], start=True, stop=True
        )
        nc.tensor.matmul(
            warm_ps[0:1, 0:1], ident_sb[:, 0:1], ones_bf[:], start=True, stop=True
        )
        nc.tensor.matmul(
            warm_ps[0:1, 0:1], w1_sb[:, 0, 0:1], ones_bf[:], start=True, stop=True
        )
        nc.tensor.matmul(
            warm_ps[0:1, 0:1], ones_bf[:], w2_sb[:], start=True, stop=True
        )

        def emit_mm4_slice(st, lo, hi):
            """Accumulate seq tiles [lo, hi) of a pending example into outacc."""
            for t in range(lo, hi):
                nc.tensor.matmul(
                    st["outacc"][:],
                    st["p_bf"][:, t : t + 1],
                    st["x_ch"][:, t, :],
                    start=(t == 0),
                    stop=(t == S_TILES - 1),
                )

        def finish_phase_b(st):
            out_sb = small_pool.tile([1, C], F32, tag="out_sb")
            nc.vector.tensor_scalar_mul(out_sb[:], st["outacc"][:], st["recip"][:])
            nc.scalar.dma_start(out=out_ext[st["b"] : st["b"] + 1, :], in_=out_sb[:])

        def emit_mm1_tanh(st2, ch):
            """mm1 + tanh for chunk ch (emitted one chunk late: the xt drains
            and the WAR-cleared hT PSUM slots are then a full chunk old, so
            the in-order PE queue never stalls on DVE/ScalarE latency)."""
            xt0, xt1 = st2["xts"][ch]
            hts = []
            for sub in range(T_PER_CH * 128 // 512):
                s0 = sub * 512
                hT_ps = psum_ht.tile([128, 512], F32, tag="hT")
                nc.tensor.matmul(
                    hT_ps[:], w1_sb[:, 0, :], xt0[:, s0 : s0 + 512],
                    start=True, stop=False,
                )
                nc.tensor.matmul(
                    hT_ps[:], w1_sb[:, 1, :], xt1[:, s0 : s0 + 512],
                    start=False, stop=True,
                )
                h_t = ht_pool.tile([128, 512], BF16, tag="ht")
                nc.scalar.activation(h_t[:], hT_ps[:], AF.Tanh, bias=b1_sb[:])
                hts.append(h_t)
            st2["hts"][ch] = hts

        def emit_mm2(st2, ch):
            """scores for chunk ch (a further half-chunk later, so each tanh
            had a whole chunk of ScalarE slack before its mm2 reads it)."""
            for sub in range(T_PER_CH * 128 // 512):
                h_t = st2["hts"][ch][sub]
                for j in range(4):
                    jj = ch * T_PER_CH + sub * 4 + j
                    nc.tensor.matmul(
                        st2["scoresT_ps"][:, jj : jj + 1],
                        h_t[:, j * 128 : (j + 1) * 128],
                        w2_sb[:],
                        start=True,
                        stop=True,
                    )

        pending = None
        for b in [bb for _ in range(reps) for bb in range(B_LOC)]:
            # One PSUM bank per example holds everything softmax-related:
            # cols [0:64] scoresT, col [64] the exp-sum, cols [65:321] the
            # output accumulator (partition 0 only for the latter two). This
            # keeps every consumer's dependencies on a single semaphore.
            scoresT_ps = psum_sc.tile([128, S_TILES + 1 + C], F32, tag="scoresT")
            # whole example's x in one slot; loaded by CH//2 batched DMAs
            # (2MB each) to amortize descriptor-gen + completion fixed costs.
            x_ch = xnat_pool.tile([128, S_TILES, C], BF16, tag="xnat")
            st2 = {"scoresT_ps": scoresT_ps, "xts": {}, "hts": {}}
            for ch in range(CH):
                # keep the PE busy across example boundaries: weave the
                # previous example's weighted-sum matmuls between chunks.
                # Emitted at chunk TOP so later same-engine waits imply them.
                if mode != "dma":
                    if pending is not None:
                        emit_mm4_slice(pending, ch * T_PER_CH, (ch + 1) * T_PER_CH)
                    if ch > 1:
                        emit_mm2(st2, ch - 2)
                if ch % 2 == 0 and mode != "pe":
                    # load 2048 seq rows, cast fp32->bf16 during DMA
                    t0g = ch * T_PER_CH
                    src = x_ext[b, t0g * 128 : (t0g + 2 * T_PER_CH) * 128, :]
                    nc.gpsimd.dma_start(
                        out=x_ch[:, t0g : t0g + 2 * T_PER_CH, :],
                        in_=src.rearrange("(t p) c -> p t c", p=128),
                    )
                if mode == "dma":
                    continue

                # TensorE transpose of each [128,128] block into bf16 PSUM,
                # per c-half; VectorE drains PSUM -> SBUF (cheap: bf16 2x mode)
                ps_xt0 = psum_xt.tile([128, T_PER_CH * 128], BF16, tag="ps_xt0")
                ps_xt1 = psum_xt.tile([128, T_PER_CH * 128], BF16, tag="ps_xt1")
                for t in range(T_PER_CH):
                    nc.tensor.transpose(
                        ps_xt0[:, t * 128 : (t + 1) * 128],
                        x_ch[:, ch * T_PER_CH + t, 0:128],
                        ident_sb[:],
                    )
                    nc.tensor.transpose(
                        ps_xt1[:, t * 128 : (t + 1) * 128],
                        x_ch[:, ch * T_PER_CH + t, 128:256],
                        ident_sb[:],
                    )
                xt0 = xt_pool.tile([128, T_PER_CH * 128], BF16, tag="xt0")
                xt1 = xt_pool.tile([128, T_PER_CH * 128], BF16, tag="xt1")
                nc.vector.tensor_copy(xt0[:], ps_xt0[:])
                nc.vector.tensor_copy(xt1[:], ps_xt1[:])
                st2["xts"][ch] = (xt0, xt1)

                if ch > 0:
                    emit_mm1_tanh(st2, ch - 1)

                if pending is not None and ch == CH - 1:
                    finish_phase_b(pending)
                    pending = None

            if mode == "dma":
                continue
            # ---- tail: last chunk's scores, then softmax reductions ----
            emit_mm1_tanh(st2, CH - 1)
            emit_mm2(st2, CH - 2)
            emit_mm2(st2, CH - 1)
            p_bf = small_pool.tile([128, S_TILES], BF16, tag="p")
            sumrow = small_pool.tile([128, 1], F32, tag="sumrow")
            nc.scalar.activation(
                p_bf[:], scoresT_ps[:, 0:S_TILES], AF.Exp, accum_out=sumrow[:]
            )
            # cross-partition exp-sum lands in the scores tile's spare column
            # (same PSUM bank -> no extra slot, and the matmul's only wait is
            # the ScalarE accum above)
            nc.tensor.matmul(
                scoresT_ps[0:1, S_TILES : S_TILES + 1],
                ones_sb[:],
                sumrow[:],
                start=True,
                stop=True,
            )
            # give the DVE an up-to-date ScalarE observation so the
            # reciprocal's only explicit wait is the PE (sum matmul)
            dve_obs = small_pool.tile([1, 1], BF16, tag="dve_obs")
            nc.vector.tensor_copy(dve_obs[:], p_bf[0:1, 0:1])
            recip = small_pool.tile([1, 1], F32, tag="recip")
            nc.vector.reciprocal(recip[:], scoresT_ps[0:1, S_TILES : S_TILES + 1])
            pending = {
                "b": b,
                "p_bf": p_bf,
                "x_ch": x_ch,
                "recip": recip,
                "outacc": scoresT_ps[0:1, S_TILES + 1 : S_TILES + 1 + C],
            }

        # tail: last example's weighted sum
        if pending is not None:
            emit_mm4_slice(pending, 0, S_TILES)
            finish_phase_b(pending)

    if strip_waits:
        _strip_implied_self_waits(nc)
    return nc


def _strip_implied_self_waits(nc: bass.Bass) -> None:
    """Reduce per-instruction sync waits to what the hardware needs.

    Walrus accepts at most ONE sync wait per engine ISA instruction, but
    Tile emits waits per logical dependency. Two sound reductions:

    1. Engine-clock elision. Each engine's sequencer evaluates waits in
       program order and engines retire in order, so if an earlier
       instruction on the SAME engine already waited for sem >= v' (v'>=v),
       a later instruction's wait for sem >= v is vacuous: the semaphore
       condition held before the predecessor issued. (Tile deliberately
       doesn't do this transitive per-proc minimization.) Also covers waits
       on the engine's own completion semaphore.

    2. x-load WAW elision. Each x-chunk load carries {PE >= k (WAR: all
       readers of the slot's old contents are done), DMASW >= v (WAW vs the
       old writer)}. The readers read-after-wrote the old data, so the WAR
       wait transitively dominates the WAW wait; drop the DMASW wait.
    """
    eng_prefix = {
        mybir.EngineType.PE: "PE_",
        mybir.EngineType.DVE: "DVE_",
        mybir.EngineType.Activation: "Activation_",
        mybir.EngineType.Pool: "Pool_",
        mybir.EngineType.SP: "SP_",
    }
    # Sems that are ever non-monotonically updated (barrier gather/release
    # use sem-sub) are excluded from all reasoning: their values regress.
    nonmono: set[str] = set()
    for f in nc.m.functions:
        for blk in f.blocks:
            for inst in blk.instructions:
                si = inst.sync_info
                if si is None:
                    continue
                for u in si.on_update:
                    if u.sync_type == "semaphore" and u.update_mode not in (
                        "sem-inc",
                        "sem-add-imm",
                    ):
                        nonmono.add(u.ant_name)

    observed: dict[mybir.EngineType, dict[str, int]] = {}
    for f in nc.m.functions:
        for blk in f.blocks:
            splits: list[tuple[int, list]] = []
            for idx, inst in enumerate(blk.instructions):
                si = inst.sync_info
                if si is None:
                    continue
                tn = type(inst).__name__
                if tn == "InstEventSemaphore":
                    continue  # barrier machinery: leave untouched
                eng = inst.engine
                obs = observed.setdefault(eng, {})
                pref = eng_prefix.get(eng)
                is_x_load = False
                if tn == "InstDMACopy" and eng == mybir.EngineType.Pool:
                    try:
                        is_x_load = "x_ch" in str(inst.outs[0])
                    except Exception:
                        is_x_load = False
                has_pe_wait = any(
                    w.sync_type == "semaphore" and w.ant_name.startswith("PE_")
                    for w in si.on_wait
                )
                kept = []
                for w in si.on_wait:
                    if (
                        w.sync_type != "semaphore"
                        or w.wait_mode != "sem-ge-imm"
                        or w.ant_name in nonmono
                        or tn == "InstDrain"
                    ):
                        kept.append(w)
                        continue
                    # (1) engine-clock / self-wait elision
                    if obs.get(w.ant_name, 0) >= w.wait_value:
                        continue
                    # (2) x-load WAW-vs-old-writer elision
                    if (
                        is_x_load
                        and has_pe_wait
                        and w.ant_name.startswith("DMASW")
                    ):
                        continue
                    kept.append(w)
                # record knowledge from ALL original waits (sound even for
                # stripped ones: the condition held at this program point)
                for w in si.on_wait:
                    if (
                        w.sync_type == "semaphore"
                        and w.wait_mode == "sem-ge-imm"
                        and w.ant_name not in nonmono
                    ):
                        if obs.get(w.ant_name, 0) < w.wait_value:
                            obs[w.ant_name] = w.wait_value
                if len(kept) != len(si.on_wait):
                    si.on_wait = kept
                    kept = si.on_wait  # re-read normalized
                if len(kept) > 1:
                    # Hardware takes one sync wait per instruction: carry the
                    # surplus on single-wait Drain instructions inserted just
                    # before (same engine => sequencer evaluates them first).
                    extras = []
                    for i, w in enumerate(kept[:-1]):
                        d = mybir.InstDrain(
                            name=f"{inst.name}-w{i}", ins=[], outs=[]
                        )
                        d.engine = inst.engine
                        d.sync_info = mybir.SyncInfo(on_wait=[w], on_update=[])
                        extras.append(d)
                    si.on_wait = [kept[-1]]
                    splits.append((idx, extras))
                # engine-own completion increments advance the engine clock.
                # Pool excluded: its 8 Q7 cores may retire out of order, so
                # completion-count knowledge is only valid for strict-FIFO
                # engines (wait-observation inheritance above is still valid
                # for Pool -- the NX sequencer evaluates waits in order).
                if pref is not None and eng != mybir.EngineType.Pool:
                    for u in si.on_update:
                        if (
                            u.sync_type == "semaphore"
                            and u.update_mode in ("sem-inc", "sem-add-imm")
                            and u.ant_name.startswith(pref)
                        ):
                            obs[u.ant_name] = obs.get(u.ant_name, 0) + (
                                u.update_value or 1
                            )
            if splits:
                il = blk.instructions
                for idx, extras in reversed(splits):
                    for d in reversed(extras):
                        il.insert(idx, d)


_NC_CACHE = None


def _get_nc() -> bass.Bass:
    global _NC_CACHE
    if _NC_CACHE is None:
        _NC_CACHE = build_nc()
    return _NC_CACHE


def make_in_maps(x, W1, b1, w2):
    import ml_dtypes

    ident = np.eye(128, dtype=ml_dtypes.bfloat16)
    return [
        {
            "x": x[i * B_LOC : (i + 1) * B_LOC],
            "W1": W1,
            "b1": b1,
            "w2": w2,
            "ident": ident,
        }
        for i in range(N_CORES)
    ]


def kernel(x, W1, b1, w2, b2=None, **_unused) -> np.ndarray:
    """Full-input entry point: shard batch across 8 cores, run, gather.

    b2 is mathematically irrelevant (softmax shift invariance) and ignored.
    """
    x = np.ascontiguousarray(np.asarray(x, dtype=np.float32))
    W1 = np.ascontiguousarray(np.asarray(W1, dtype=np.float32))
    b1 = np.ascontiguousarray(np.asarray(b1, dtype=np.float32))
    w2 = np.ascontiguousarray(np.asarray(w2, dtype=np.float32))
    assert x.shape == (B, S, C), x.shape

    nc = _get_nc()
    in_maps = make_in_maps(x, W1, b1, w2)
    res = run_bass_kernel_spmd(nc, in_maps, list(range(N_CORES))).results
    out = np.concatenate([res[i]["out"] for i in range(N_CORES)], axis=0)
    return out.astype(np.float32)

